# revision 18
# baseline (speedup 1.0000x reference)
"""CRF negative-log-likelihood loss kernel for Trainium2 (8 NeuronCores).

Strategy (v2: 4-way time-split, 128-round serial depth)
-------------------------------------------------------
Data-parallel over the batch: 32 sequences -> 4 per core. The log-partition
chain Z = 1^T A_511 ... A_1 e_0 (A_t = diag(e_t) M, M = exp(T)) is split into
four 128-step time chunks:

  - chunk [1,128):   forward vector chain  alpha_t = e_t * (M @ alpha_{t-1})
  - chunk [384,512): backward vector chain v_t = e_t * (M^T @ v_{t+1})
  - chunks [128,256) and [256,384): interior operators S = A_hi ... A_lo,
    computed as their transposes X = S^T by a basis chain
    Y <- e_t * (M^T @ Y) (t decreasing, Y seeded with diag(e_hi)), so that
    matmul(lhsT=X, rhs) applies S directly at combine time.

That cuts the serial dependency depth from 255 to ~128 rounds. Per round a
core runs 5 independent chains: one paired [alpha; v] vector chain ([128,4]
state, block-diag weights) and four interior stacks ([128,64] states holding
seq b's S1 chain on partitions 0-63 and its S2 chain on 64-127, block-diag
exp(T) weights). Interior per-step emission multiplies are per-partition
tensor_scalar ops spread across Vector, Scalar(Act) and GpSimd engines so no
engine exceeds the ~480ns hop latency.

All emissions are prescaled by 2^-7.5 (folded into the on-chip Exp bias);
magnitudes then stay inside fp32/bf16 range for the whole 512 steps, so the
kernel needs NO renormalization events. The host packs every emission operand
the chip will touch into one [128, 1028] fp32 stream (exp'd on-chip in 4
chunks); there are no on-chip transposes.

The gold-path score reuses the one-hot matmul scheme of v1, forced after the
recurrence.
"""

import math

import numpy as np
from contextlib import ExitStack

B, T_LEN, L = 32, 512, 64
N_CORES = 8
BPC = B // N_CORES  # sequences per core
R = 128             # rounds (chunk length)
PRE_BITS = 7.5      # emission prescale 2^-PRE_BITS, folded into Exp bias
NCOL = 4 + 8 * R    # seed cols + 8 cols per round

_compiled = None


def _build_program():
    import concourse.bacc as bacc
    import concourse.tile as tile
    import concourse.mybir as mybir
    from concourse.alu_op_type import AluOpType

    f32 = mybir.dt.float32
    bf16 = mybir.dt.bfloat16
    Af = mybir.ActivationFunctionType

    nc = bacc.Bacc("TRN2", target_bir_lowering=False, debug=False,
                   num_devices=N_CORES)

    eops_d = nc.dram_tensor("eops", [128, NCOL], f32,
                            kind="ExternalInput").ap()
    feats_d = nc.dram_tensor("feats", [BPC * T_LEN, L], f32,
                             kind="ExternalInput").ap()
    oh_d = nc.dram_tensor("oh", [BPC * (T_LEN + 1), L], f32,
                          kind="ExternalInput").ap()
    tt_d = nc.dram_tensor("tt", [L, L], f32, kind="ExternalInput").ap()
    mask_d = nc.dram_tensor("mask", [2 * L, L], f32, kind="ExternalInput").ap()
    i2_d = nc.dram_tensor("i2", [128, L], f32, kind="ExternalInput").ap()
    out_d = nc.dram_tensor("out", [1, BPC], f32, kind="ExternalOutput").ap()

    EXP_BIAS = float(-PRE_BITS * math.log(2.0))
    LN_OFF = float(T_LEN * PRE_BITS * math.log(2.0))

    with tile.TileContext(nc) as tc, ExitStack() as ctx:
        consts = ctx.enter_context(tc.tile_pool(name="consts", bufs=1))
        goldp = ctx.enter_context(tc.tile_pool(name="gold", bufs=16))
        vstate = ctx.enter_context(tc.tile_pool(name="vstate", bufs=3))
        ypools = [ctx.enter_context(tc.tile_pool(name=f"y{s}", bufs=3))
                  for s in range(4)]
        vtmp = ctx.enter_context(tc.tile_pool(name="vtmp", bufs=8))
        vq = ctx.enter_context(tc.tile_pool(name="vq", bufs=1, space="PSUM"))
        pps = [ctx.enter_context(tc.tile_pool(name=f"p{s}", bufs=1,
                                              space="PSUM"))
               for s in range(4)]
        cps = ctx.enter_context(tc.tile_pool(name="cps", bufs=2, space="PSUM"))
        tpp = ctx.enter_context(tc.tile_pool(name="tpsum", bufs=1,
                                             space="PSUM"))

        # ---- constants ----
        ones128 = consts.tile([128, 1], f32)
        nc.gpsimd.memset(ones128[:], 1.0)
        onesb = consts.tile([L, 1], bf16)
        nc.gpsimd.memset(onesb[:], 1.0)
        ttile = consts.tile([L, L], f32)          # T^T
        nc.sync.dma_start(out=ttile[:], in_=tt_d)
        tstr = consts.tile([L, L], f32)           # T (straight)
        nc.sync.dma_start(out=tstr[:], in_=mask_d[L:2 * L, :])
        i2f = consts.tile([128, L], f32)          # [I64; I64]
        nc.sync.dma_start(out=i2f[:], in_=i2_d)
        mask_sb = consts.tile([128, L], f32)
        nc.sync.dma_start(out=mask_sb[:], in_=mask_d)
        ebias = consts.tile([128, 1], f32)
        nc.gpsimd.memset(ebias[:], EXP_BIAS)

        # Wc: paired vector-chain weights. cols 0-63 = [exp(T^T); 0]
        # (fwd M-contraction), cols 64-127 = [0; exp(T)] (bwd M^T).
        Wc = consts.tile([128, 128], bf16)
        nc.gpsimd.memset(Wc[:], 0.0)
        nc.scalar.activation(Wc[0:L, 0:L], ttile[:], Af.Exp)
        nc.scalar.activation(Wc[L:128, L:128], tstr[:], Af.Exp)
        # Wd: interior stack weights, block-diag(exp(T), exp(T)): both halves
        # perform the M^T contraction out = M^T @ Y.
        Wd = consts.tile([128, 128], bf16)
        nc.gpsimd.memset(Wd[:], 0.0)
        nc.scalar.activation(Wd[0:L, 0:L], tstr[:], Af.Exp)
        nc.scalar.activation(Wd[L:128, L:128], tstr[:], Af.Exp)

        # ---- emission stream: DMA fp32, Exp(+bias) in 4 chunks ----
        # cols [0,516) = seeds + vector-chain operands -> bf16 (matmul/TT);
        # cols [516,1028) = interior ts scalars -> fp32 (tensor_scalar
        # requires fp32 scalar operands).
        eall_v = consts.tile([128, 516], bf16)
        eall_s = consts.tile([128, 512], f32)
        for k, (a, bnd) in enumerate([(0, 260), (516, 772),
                                      (260, 516), (772, NCOL)]):
            stg = consts.tile([128, bnd - a], f32, tag=f"estg{k}")
            nc.gpsimd.dma_start(out=stg[:], in_=eops_d[:, a:bnd])
            dst = (eall_v[:, a:bnd] if bnd <= 516
                   else eall_s[:, a - 516:bnd - 516])
            nc.scalar.activation(dst, stg[:], Af.Exp, bias=ebias[:])

        def veccol(r):
            return eall_v[:, 4 + 4 * r: 8 + 4 * r]

        def stackcol(r, s):
            c = 4 * r + s
            return eall_s[:, c:c + 1]

        # ---- 128 rounds: 1 paired vector chain + 4 interior stacks ----
        # ts engine assignment: stack0 -> DVE, stacks 1,2 -> Act, stack3 ->
        # GpSimd; the vector TT also rides DVE.
        state = eall_v[:, 0:4]  # seeds [alpha_0; v_511]
        Y = [None] * 4
        for s in range(4):
            y0 = ypools[s].tile([128, L], bf16, tag=f"y{s}")
            if s in (0, 3):
                nc.vector.tensor_scalar_mul(y0[:], i2f[:], stackcol(0, s))
            else:
                nc.scalar.activation(y0[:], i2f[:], Af.Copy,
                                     scale=stackcol(0, s))
            Y[s] = y0

        for r in range(1, R + 1):
            # vector chain: rounds 1..127 consume e_{r} / e_{511-r}
            if r <= R - 1:
                q = vq.tile([128, 4], f32, tag="q")
                nc.tensor.matmul(q[:], lhsT=Wc[:], rhs=state,
                                 start=True, stop=True)
                ns = vstate.tile([128, 4], bf16, tag="vs")
                nc.vector.tensor_tensor(ns[:], q[:], veccol(r - 1),
                                        op=AluOpType.mult)
                state = ns[:]
            for s in range(4):
                p = pps[s].tile([128, L], f32, tag=f"p{s}")
                nc.tensor.matmul(p[:], lhsT=Wd[:], rhs=Y[s][:],
                                 start=True, stop=True)
                if r <= R - 1:
                    yn = ypools[s].tile([128, L], bf16, tag=f"y{s}")
                    if s in (0, 3):
                        nc.vector.tensor_scalar_mul(yn[:], p[:],
                                                    stackcol(r, s))
                    else:
                        nc.scalar.activation(yn[:], p[:], Af.Copy,
                                             scale=stackcol(r, s))
                    Y[s] = yn
                else:
                    # r == R: final X = M^T Y_127 -> SBUF bf16
                    xs = vtmp.tile([128, L], bf16, tag=f"x{s}")
                    nc.vector.tensor_copy(xs[:], p[:])
                    Y[s] = xs

        # ---- combine ----
        # partition-aligned copies of the bottom halves (X2 of each stack,
        # and v from the vector state) via SBUF->SBUF DMA
        X2 = []
        for b in range(4):
            x2 = vtmp.tile([L, L], bf16, tag=f"x2{b}")
            nc.sync.dma_start(out=x2[:], in_=Y[b][L:128, :])
            X2.append(x2)

        # u = M^T v_384: contract the full [alpha; v] state against the
        # [0; M] column block so no partition-offset operands are needed.
        ups = cps.tile([L, 4], f32, tag="c")
        nc.tensor.matmul(ups[:], lhsT=Wd[:, L:128], rhs=state,
                         start=True, stop=True)
        usb = vtmp.tile([L, 4], bf16, tag="usb")
        nc.vector.tensor_copy(usb[:], ups[:])

        g = vtmp.tile([L, 4], bf16, tag="g")
        for b in range(4):
            z1 = cps.tile([L, 1], f32, tag="c")
            nc.tensor.matmul(z1[:], lhsT=Y[b][0:L, :],
                             rhs=state[0:L, b:b + 1], start=True, stop=True)
            z1s = vtmp.tile([L, 1], bf16, tag=f"z1s{b}")
            nc.vector.tensor_copy(z1s[:], z1[:])
            z2 = cps.tile([L, 1], f32, tag="c")
            nc.tensor.matmul(z2[:], lhsT=X2[b][:], rhs=z1s[:],
                             start=True, stop=True)
            nc.vector.tensor_mul(g[:, b:b + 1], z2[:], usb[:, b:b + 1])
        zrow = cps.tile([1, 4], f32, tag="c")
        zi = nc.tensor.matmul(zrow[:], lhsT=onesb[:], rhs=g[:],
                              start=True, stop=True)
        lnz = vtmp.tile([1, 4], f32, tag="lnz")
        nc.scalar.activation(lnz[:], zrow[:], Af.Ln)

        # ---- gold score via one-hot matmuls, forced after the loop ----
        from concourse.tile_rust import add_dep_helper
        feats_bmaj = feats_d.rearrange("(b t) l -> b t l", b=BPC)
        Vt = consts.tile([128, BPC], f32)
        for b in range(BPC):
            gps = tpp.tile([128, L], f32, tag="tp")
            for c in range(4):
                o0 = b * (T_LEN + 1) + c * 128
                cat = goldp.tile([128, 128], f32, tag="cat")
                nc.sync.dma_start(
                    out=cat[:, 0:L],
                    in_=feats_bmaj[b, c * 128:(c + 1) * 128, :])
                nc.sync.dma_start(out=cat[:, L:2 * L],
                                  in_=oh_d[o0 + 1:o0 + 129, :])
                ohp = goldp.tile([128, L], f32, tag="ohp")
                nc.sync.dma_start(out=ohp[:], in_=oh_d[o0:o0 + 128, :])
                gi = nc.tensor.matmul(gps[:], lhsT=cat[:], rhs=ohp[:],
                                      start=(c == 0), stop=(c == 3))
                add_dep_helper(gi.ins, zi.ins, sync=True,
                               reason="gold matmuls after combine")
            gsc = vtmp.tile([128, L], f32, tag="gsc")
            nc.vector.tensor_mul(gsc[:], gps[:], mask_sb[:])
            nc.vector.tensor_reduce(Vt[:, b:b + 1], gsc[:],
                                    axis=mybir.AxisListType.X,
                                    op=AluOpType.add)
        gold_ps = tpp.tile([128, L], f32, tag="tp")
        nc.tensor.matmul(gold_ps[0:1, 0:BPC], lhsT=ones128[:, 0:1], rhs=Vt[:],
                         start=True, stop=True)

        res0 = vtmp.tile([1, BPC], f32, tag="res0")
        nc.vector.tensor_tensor(res0[:], lnz[:], gold_ps[0:1, 0:BPC],
                                op=AluOpType.subtract)
        res = vtmp.tile([1, BPC], f32, tag="res")
        nc.vector.tensor_scalar_add(res[:], res0[:], LN_OFF)
        nc.sync.dma_start(out=out_d, in_=res[:])

    import concourse.bacc as bacc2
    orig = bacc2.Bacc.move_matmul_waits_to_ldweights
    if SKIP_LDW_WAIT_PASS:
        bacc2.Bacc.move_matmul_waits_to_ldweights = lambda self: None
    try:
        nc.compile()
    finally:
        bacc2.Bacc.move_matmul_waits_to_ldweights = orig
    return nc


SKIP_LDW_WAIT_PASS = True


def _prep_in_maps(feats, tags, T):
    feats = np.ascontiguousarray(np.asarray(feats, dtype=np.float32))
    T_np = np.ascontiguousarray(np.asarray(T, dtype=np.float32))
    tags_np = np.asarray(tags).astype(np.int64)

    oh = np.zeros((B, T_LEN + 1, L), dtype=np.float32)
    oh[np.arange(B)[:, None], np.arange(T_LEN)[None, :], tags_np] = 1.0
    mask_const = np.concatenate([np.eye(L, dtype=np.float32), T_np], axis=0)
    tt = np.ascontiguousarray(T_np.T)
    i2 = np.concatenate([np.eye(L, dtype=np.float32)] * 2, axis=0)

    r7 = np.arange(R - 1)
    r8 = np.arange(R)
    in_maps = []
    for c in range(N_CORES):
        sl = slice(c * BPC, (c + 1) * BPC)
        fb = feats[sl]  # [4, 512, 64]
        eops = np.zeros((128, NCOL), dtype=np.float32)
        for b in range(BPC):
            eops[0:L, b] = fb[b, 0]
            eops[L:128, b] = fb[b, T_LEN - 1]
            eops[0:L, 4 + 4 * r7 + b] = fb[b, 1 + r7].T
            eops[L:128, 4 + 4 * r7 + b] = fb[b, T_LEN - 2 - r7].T
            eops[0:L, 516 + 4 * r8 + b] = fb[b, 2 * R - 1 - r8].T
            eops[L:128, 516 + 4 * r8 + b] = fb[b, 3 * R - 1 - r8].T
        in_maps.append({
            "eops": eops,
            "feats": np.ascontiguousarray(fb.reshape(BPC * T_LEN, L)),
            "oh": np.ascontiguousarray(oh[sl].reshape(BPC * (T_LEN + 1), L)),
            "tt": tt,
            "mask": mask_const,
            "i2": i2,
        })
    return in_maps


def kernel(feats, tags, T):
    global _compiled
    from concourse.bass_utils import run_bass_kernel_spmd

    if _compiled is None:
        _compiled = _build_program()
    nc = _compiled

    in_maps = _prep_in_maps(feats, tags, T)
    res = run_bass_kernel_spmd(nc, in_maps, list(range(N_CORES)))
    out = np.concatenate(
        [res.results[c]["out"].reshape(BPC) for c in range(N_CORES)])
    return out.astype(np.float32)
